# revision 39
# baseline (speedup 1.0000x reference)
"""LocalWindowAttention Trainium2 kernel (Bass/Tile), 8-core SPMD.

Problem: x[B=4, S=4096, E=512] -> out[B, S, E]
  qkv = x @ W_qkv + b_qkv ; q,k,v = split(qkv)
  scores = (q @ k.T) / sqrt(E), banded mask |i-j| <= 64, softmax
  out = (attn @ v) @ W_out + b_out

Sharding: 8 cores = (batch b in 0..3) x (seq half h in 0..1). Each core owns
2048 query rows and loads a 64-row halo of x on each side (zero-padded at
sequence boundaries) -- no collectives.

Algebraic restructure (host-side weight fusion; softmax invariances):
  scores_ij = q_i . k_j / sqrt(E)
            = x_i (Wq Wk^T / sqrt(E)) x_j^T + x_j . (Wk bq / sqrt(E)) + f(i)
  where f(i) terms are constant across the softmax axis and drop. So with
  G = Wq Wk^T / sqrt(E) and u = Wk bq / sqrt(E):
    g = x G + u   (one [S,E]x[E,E] GEMM; k-projection is just x itself)
    scores = g x^T
  Softmax rows sum to 1, so the v/out projections fuse:
    out = attn @ (x (Wv Wout) + (bv Wout + bout)) = attn @ vt
  with Wt = Wv Wout and bt = bv Wout + bout precomputed on host. This
  removes the k-projection, the whole output projection, and both bias
  matmuls: per-core PE work drops from ~190k to ~104k PE columns.

All matmuls run in bf16 (1 PE cycle/column incl. transposes; inputs rounded
on host; fp8 DoubleRow was measured slower than bf16 on HW despite the cost
model). Softmax normalization is applied late (per-partition scale on the
attended output) so the raw exp output feeds the PE transpose directly; the
output is written bf16 and upcast on host.

Schedule: head DMAs split across the SP and Activation queues (DMA issue is
~0.65us per instruction per queue) with x packed [p, e, n] so one DMA per
column-slice covers all 4 contraction chunks. The attention loop is
software-pipelined two stages deep so the PE never waits on the
vector/scalar exp chain, with one vt row-chunk per iteration as extra PE
slack. Measured ~69.1us on HW vs the 134.8us fp32r baseline.
"""

import sys

sys.path.insert(0, "/opt/trn_rl_repo")

import numpy as np
import ml_dtypes

import concourse.bass as bass  # noqa: F401  (registers types)
import concourse.tile as tile
from concourse import bacc, mybir
from concourse.bass_utils import run_bass_kernel_spmd

F32 = mybir.dt.float32
BF16 = mybir.dt.bfloat16
BF16_NP = ml_dtypes.bfloat16

B, S, E = 4, 4096, 512
WINDOW = 64
HALF = S // 2              # 2048 query rows per core
ROWS = HALF + 2 * WINDOW   # 2176 local rows incl. halo
EC = E // 128              # 4 contraction chunks
NT = HALF // 128           # 16 query subtiles per core
NV = ROWS // 128           # 17 vt row chunks

_NC_CACHE = {}


def _build():
    nc = bacc.Bacc("TRN2", target_bir_lowering=False, debug=False, num_devices=8)

    # x packed [p, e, n] so one DMA per column-slice covers all 4 chunks
    xT_d = nc.dram_tensor("xTp", [128, EC, ROWS], BF16, kind="ExternalInput")
    # G's f0 column blocks packed [p, e, m] (the first matmul group's
    # weights in a single 0.125MB DMA with 1KB lines); f1..f3 per chunk
    g0_d = nc.dram_tensor("g0", [128, EC, 128], BF16, kind="ExternalInput")
    g_d = nc.dram_tensor("gmat", [E, E], BF16, kind="ExternalInput")
    wt_d = nc.dram_tensor("wt", [E, E], BF16, kind="ExternalInput")
    u_d = nc.dram_tensor("ubias", [128, EC], F32, kind="ExternalInput")
    bt_d = nc.dram_tensor("btb", [128, E], F32, kind="ExternalInput")
    mask_d = nc.dram_tensor("masks", [128, 3 * 256], F32, kind="ExternalInput")
    id_d = nc.dram_tensor("ident", [128, 128], BF16, kind="ExternalInput")
    out_d = nc.dram_tensor("out", [HALF, E], BF16, kind="ExternalOutput")

    ACT = mybir.ActivationFunctionType

    with tile.TileContext(nc) as tc:
        with (
            tc.tile_pool(name="const", bufs=1) as const,
            tc.tile_pool(name="big", bufs=1) as big,
        ):
            # ---- constants ----
            g0_sb = const.tile([128, EC, 128], BF16, name="gf0", tag="gf0")
            g_sb = [const.tile([128, E], BF16, name=f"g{e}", tag=f"g{e}")
                    for e in range(EC)]
            wt_sb = [const.tile([128, E], BF16, name=f"wt{e}", tag=f"wt{e}")
                     for e in range(EC)]
            u_sb = const.tile([128, EC], F32, name="ub", tag="ub")
            bt_sb = const.tile([128, E], F32, name="btb", tag="btb")
            mask_sb = const.tile([128, 3 * 256], F32, name="msk", tag="msk")
            id_sb = const.tile([128, 128], BF16, name="idn", tag="idn")

            # ---- persistent tensors ----
            xTa = big.tile([128, EC, ROWS], BF16, name="xTa", tag="xTa")
            gT = [big.tile([128, HALF], BF16, name=f"gT{f}", tag=f"gT{f}")
                  for f in range(EC)]
            vt = [big.tile([128, E], BF16, name=f"vt{r}", tag=f"vt{r}")
                  for r in range(NV)]

            # Head DMAs spread across four engine queues (each DMA_DIRECT2D
            # occupies its issuing queue ~0.65us, so serialization on Sync
            # was the old critical path). The first matmul group needs only
            # g0 (0.125MB, gpsimd queue) + xT cols 0..192 (0.19MB, sync).
            XSL = [(0, 384), (384, 640), (640, 1152), (1152, 1664),
                   (1664, ROWS)]
            for c0, c1 in XSL:
                nc.sync.dma_start(out=xTa[:, :, c0:c1],
                                  in_=xT_d[:, :, c0:c1])
            nc.scalar.dma_start(out=g0_sb, in_=g0_d[:, :, :])
            nc.scalar.dma_start(out=u_sb, in_=u_d[:, :])
            for e in range(EC):
                nc.scalar.dma_start(out=g_sb[e][:, 128:512],
                                    in_=g_d[128 * e:128 * (e + 1), 128:512])
            for e in range(EC):
                nc.sync.dma_start(out=wt_sb[e],
                                  in_=wt_d[128 * e:128 * (e + 1), :])
            nc.sync.dma_start(out=bt_sb, in_=bt_d[:, :])
            nc.sync.dma_start(out=mask_sb, in_=mask_d[:, :])
            nc.sync.dma_start(out=id_sb, in_=id_d[:, :])

            with tc.tile_pool(name="pp", bufs=2, space="PSUM") as pp:
                # g-projection: gT[f][:, c] = sum_e G[e,f-chunk]^T xT[e][:, 64+c]
                # (queries only: local rows 64..2112), slice-major so early
                # query columns finish across all f first; slice widths grow
                # so the PE starts while x is still streaming in.
                GSL = [(0, 320), (320, 256), (576, 512), (1088, 512),
                       (1600, 448)]
                for s0, ns in GSL:
                    for f in range(EC):
                        ps = pp.tile([128, 512], F32, name=f"pg{f}_{s0}", tag="pp")
                        for e in range(EC):
                            lhs = (g0_sb[:, e, :] if f == 0
                                   else g_sb[e][:, 128 * f:128 * (f + 1)])
                            nc.tensor.matmul(
                                ps[:, :ns],
                                lhs,
                                xTa[:, e, 64 + s0:64 + s0 + ns],
                                start=(e == 0), stop=(e == EC - 1),
                            )
                        nc.scalar.activation(
                            out=gT[f][:, s0:s0 + ns], in_=ps[:, :ns],
                            func=ACT.Identity, bias=u_sb[:, f:f + 1],
                        )

                def vt_chunk(r):
                    # vt[r] = xT[:, rows r]^T Wt + bt   (natural [rows, feat])
                    ps = pp.tile([128, 512], F32, name=f"pv{r}", tag="pp")
                    for e in range(EC):
                        nc.tensor.matmul(
                            ps[:],
                            xTa[:, e, 128 * r:128 * (r + 1)],
                            wt_sb[e][:],
                            start=(e == 0), stop=(e == EC - 1),
                        )
                    nc.vector.tensor_add(vt[r][:], ps[:], bt_sb[:])

                vt_chunk(0)

                with (
                    tc.tile_pool(name="attn", bufs=3) as attn,
                    tc.tile_pool(name="ps_s", bufs=2, space="PSUM") as ps_s,
                    tc.tile_pool(name="ps_t", bufs=2, space="PSUM") as ps_t,
                    tc.tile_pool(name="ps_a", bufs=2, space="PSUM") as ps_a,
                ):
                    # two-stage software pipeline over the 16 query tiles
                    stage = []  # holds (t, pe_t, rd)

                    def drain(stage_state):
                        t, pe_t, rd = stage_state
                        pT = []
                        for half in (0, 1):
                            tp = ps_t.tile([128, 128], BF16,
                                           name=f"tp{t}_{half}", tag="ps_t")
                            nc.tensor.transpose(
                                tp[:], pe_t[:, 128 * half:128 * (half + 1)],
                                id_sb[:])
                            sb = attn.tile([128, 128], BF16,
                                           name=f"pT{t}_{half}", tag=f"pT{half}")
                            nc.vector.tensor_copy(sb[:], tp[:])
                            pT.append(sb)
                        po = ps_a.tile([128, 512], F32, name=f"po{t}", tag="ps_a")
                        for kc in (0, 1):
                            nc.tensor.matmul(
                                po[:], pT[kc][:], vt[t + kc][:],
                                start=(kc == 0), stop=(kc == 1),
                            )
                        ost = attn.tile([128, 512], BF16, name=f"o{t}", tag="ost")
                        nc.vector.tensor_scalar_mul(ost[:], po[:], rd[:, 0:1])
                        nc.sync.dma_start(
                            out=out_d[128 * t:128 * (t + 1), :], in_=ost[:])

                    for t in range(NT):
                        vt_chunk(t + 1)
                        # scores for tile t: [128 q, 256 keys]
                        ps = ps_s.tile([128, 256], F32, name=f"s{t}", tag="ps_s")
                        for e in range(EC):
                            nc.tensor.matmul(
                                ps[:],
                                gT[e][:, 128 * t:128 * (t + 1)],
                                xTa[:, e, 128 * t:128 * t + 256],
                                start=(e == 0), stop=(e == EC - 1),
                            )
                        mi = 0 if t == 0 else (2 if t == NT - 1 else 1)
                        sm = attn.tile([128, 256], F32, name=f"sm{t}", tag="sm")
                        nc.vector.tensor_add(
                            sm[:], ps[:], mask_sb[:, 256 * mi:256 * (mi + 1)])
                        pe_t = attn.tile([128, 256], BF16, name=f"pe{t}", tag="pe")
                        rs = attn.tile([128, 1], F32, name=f"rs{t}", tag="rs")
                        nc.scalar.activation(out=pe_t[:], in_=sm[:], func=ACT.Exp,
                                             accum_out=rs[:])
                        rd = attn.tile([128, 1], F32, name=f"rd{t}", tag="rd")
                        nc.vector.reciprocal(rd[:], rs[:])
                        if len(stage) == 2:
                            drain(stage.pop(0))
                        stage.append((t, pe_t, rd))
                    for st in stage:
                        drain(st)
    nc.compile()
    return nc


def _get_nc():
    if "nc" not in _NC_CACHE:
        _NC_CACHE["nc"] = _build()
    return _NC_CACHE["nc"]


def _prep_shared(W_qkv, b_qkv, W_out, b_out):
    scale = np.float32(1.0) / np.sqrt(np.float32(E))
    Wq = np.asarray(W_qkv[:, :E], dtype=np.float32)
    Wk = np.asarray(W_qkv[:, E:2 * E], dtype=np.float32)
    Wv = np.asarray(W_qkv[:, 2 * E:], dtype=np.float32)
    bq = np.asarray(b_qkv[:E], dtype=np.float32)
    bv = np.asarray(b_qkv[2 * E:], dtype=np.float32)
    G = (Wq @ Wk.T) * scale                       # [E, E]
    u = (Wk @ bq) * scale                         # [E]
    Wt = Wv @ np.asarray(W_out, dtype=np.float32)  # [E, E]
    bt = bv @ np.asarray(W_out, dtype=np.float32) + np.asarray(
        b_out, dtype=np.float32)                  # [E]
    Gb = G.astype(BF16_NP)
    shared = {
        "gmat": np.ascontiguousarray(Gb),
        "g0": np.ascontiguousarray(
            Gb.reshape(EC, 128, E)[:, :, :128].transpose(1, 0, 2)),
        "wt": np.ascontiguousarray(Wt.astype(BF16_NP)),
        "ubias": np.ascontiguousarray(u.reshape(EC, 128).T.astype(np.float32)),
        "btb": np.ascontiguousarray(
            np.broadcast_to(bt.reshape(1, E), (128, E)).astype(np.float32)),
        "ident": np.eye(128, dtype=np.float32).astype(BF16_NP),
    }
    return shared


def _masks_for(h: int) -> np.ndarray:
    """Additive masks: 0 where attendable, -1e30 outside the band (or past
    the sequence boundary). Columns: [t0 mask | interior mask | t15 mask]."""
    ii = np.arange(128)[:, None]
    jj = np.arange(256)[None, :]
    band = (jj - ii >= 0) & (jj - ii <= 2 * WINDOW)
    m_mid = band
    m_t0 = band & (jj >= 64) if h == 0 else band
    m_t15 = band & (jj < 192) if h == 1 else band
    stacked = np.concatenate([m_t0, m_mid, m_t15], axis=1)
    return np.ascontiguousarray(
        np.where(stacked, np.float32(0.0), np.float32(-1e30)))


def _install_ntff_shim():
    """The agent image's antenv lacks axon_hooks; synthesize it from the
    boot module's ctypes NTFF driver so trace=True can capture HW timing."""
    import types
    if "antenv.axon_hooks" in sys.modules:
        return
    try:
        from trn_agent_boot.trn_boot import _ntff_profile_via_ctypes
        hook = _ntff_profile_via_ctypes("/opt/axon/libaxon_pjrt.so")
    except Exception:
        hook = None
    mod = types.ModuleType("antenv.axon_hooks")
    mod.get_axon_ntff_profile_hook = lambda: hook
    mod.set_axon_ntff_profile_hook = lambda h: None
    sys.modules["antenv.axon_hooks"] = mod
    # avoid S3 artifact upload attempts during local profile processing
    try:
        from concourse import bass_utils as _bu
        _bu.upload_artifacts = lambda tmpdir: tmpdir
    except Exception:
        pass


def kernel(x, W_qkv, b_qkv, W_out, b_out, _trace=False):
    x = np.asarray(x, dtype=np.float32)
    nc = _get_nc()
    shared = _prep_shared(W_qkv, b_qkv, W_out, b_out)
    masks = [_masks_for(0), _masks_for(1)]

    in_maps = []
    for core in range(8):
        b, h = divmod(core, 2)
        lo = h * HALF - WINDOW
        hi = lo + ROWS
        xh = np.zeros((ROWS, E), dtype=np.float32)
        s0, s1 = max(lo, 0), min(hi, S)
        xh[s0 - lo:s1 - lo] = x[b, s0:s1]
        xTp = xh.T.astype(BF16_NP).reshape(EC, 128, ROWS).transpose(1, 0, 2)
        in_maps.append({
            "xTp": np.ascontiguousarray(xTp),
            "masks": masks[h],
            **shared,
        })

    kwargs = {}
    if _trace:
        _install_ntff_shim()
        kwargs = dict(trace=True, trace_cores=[0])
    res = run_bass_kernel_spmd(nc, in_maps, core_ids=list(range(8)), **kwargs)

    out = np.empty((B, S, E), dtype=np.float32)
    for core in range(8):
        b, h = divmod(core, 2)
        out[b, h * HALF:(h + 1) * HALF] = res.results[core]["out"].astype(
            np.float32)
    if _trace:
        return out, res
    return out


# revision 40
# speedup vs baseline: 1.0624x; 1.0624x over previous
"""LocalWindowAttention Trainium2 kernel (Bass/Tile), 8-core SPMD.

Problem: x[B=4, S=4096, E=512] -> out[B, S, E]
  qkv = x @ W_qkv + b_qkv ; q,k,v = split(qkv)
  scores = (q @ k.T) / sqrt(E), banded mask |i-j| <= 64, softmax
  out = (attn @ v) @ W_out + b_out

Sharding: 8 cores = (batch b in 0..3) x (seq half h in 0..1). Each core owns
2048 query rows and loads a 64-row halo of x on each side (zero-padded at
sequence boundaries) -- no collectives.

Algebraic restructure (host-side weight fusion; softmax invariances):
  scores_ij = q_i . k_j / sqrt(E)
            = x_i (Wq Wk^T / sqrt(E)) x_j^T + x_j . (Wk bq / sqrt(E)) + f(i)
  where f(i) terms are constant across the softmax axis and drop. So with
  G = Wq Wk^T / sqrt(E) and u = Wk bq / sqrt(E):
    g = x G + u   (one [S,E]x[E,E] GEMM; k-projection is just x itself)
    scores = g x^T
  Softmax rows sum to 1, so the v/out projections fuse:
    out = attn @ (x (Wv Wout) + (bv Wout + bout)) = attn @ vt
  with Wt = Wv Wout and bt = bv Wout + bout precomputed on host. This
  removes the k-projection, the whole output projection, and both bias
  matmuls: per-core PE work drops from ~190k to ~104k PE columns.

All matmuls run in bf16 (1 PE cycle/column incl. transposes; inputs rounded
on host; fp8 DoubleRow was measured slower than bf16 on HW despite the cost
model). Softmax normalization is applied late (per-partition scale on the
attended output) so the raw exp output feeds the PE transpose directly; the
output is written bf16 and upcast on host.

Schedule: head DMAs split across the SP and Activation queues (DMA issue is
~0.65us per instruction per queue) with x packed [p, e, n] so one DMA per
column-slice covers all 4 contraction chunks. The attention loop is
software-pipelined two stages deep so the PE never waits on the
vector/scalar exp chain, with one vt row-chunk per iteration as extra PE
slack. Measured ~69.1us on HW vs the 134.8us fp32r baseline.
"""

import sys

sys.path.insert(0, "/opt/trn_rl_repo")

import numpy as np
import ml_dtypes

import concourse.bass as bass  # noqa: F401  (registers types)
import concourse.tile as tile
from concourse import bacc, mybir
from concourse.bass_utils import run_bass_kernel_spmd

F32 = mybir.dt.float32
BF16 = mybir.dt.bfloat16
BF16_NP = ml_dtypes.bfloat16

B, S, E = 4, 4096, 512
WINDOW = 64
HALF = S // 2              # 2048 query rows per core
ROWS = HALF + 2 * WINDOW   # 2176 local rows incl. halo
EC = E // 128              # 4 contraction chunks
NT = HALF // 128           # 16 query subtiles per core
NV = ROWS // 128           # 17 vt row chunks

_NC_CACHE = {}


def _build():
    nc = bacc.Bacc("TRN2", target_bir_lowering=False, debug=False, num_devices=8)

    # x packed [p, e, n] so one DMA per column-slice covers all 4 chunks
    xT_d = nc.dram_tensor("xTp", [128, EC, ROWS], BF16, kind="ExternalInput")
    # G's f0 column blocks packed [p, e, m] (the first matmul group's
    # weights in a single 0.125MB DMA with 1KB lines); f1..f3 per chunk
    g0_d = nc.dram_tensor("g0", [128, EC, 128], BF16, kind="ExternalInput")
    g_d = nc.dram_tensor("gmat", [E, E], BF16, kind="ExternalInput")
    wt_d = nc.dram_tensor("wt", [E, E], BF16, kind="ExternalInput")
    u_d = nc.dram_tensor("ubias", [128, EC], F32, kind="ExternalInput")
    bt_d = nc.dram_tensor("btb", [128, E], F32, kind="ExternalInput")
    mask_d = nc.dram_tensor("masks", [128, 3 * 256], F32, kind="ExternalInput")
    id_d = nc.dram_tensor("ident", [128, 128], BF16, kind="ExternalInput")
    out_d = nc.dram_tensor("out", [HALF, E], BF16, kind="ExternalOutput")

    ACT = mybir.ActivationFunctionType

    with tile.TileContext(nc) as tc:
        with (
            tc.tile_pool(name="const", bufs=1) as const,
            tc.tile_pool(name="big", bufs=1) as big,
        ):
            # ---- constants ----
            g0_sb = const.tile([128, EC, 128], BF16, name="gf0", tag="gf0")
            g_sb = [const.tile([128, E], BF16, name=f"g{e}", tag=f"g{e}")
                    for e in range(EC)]
            wt_sb = [const.tile([128, E], BF16, name=f"wt{e}", tag=f"wt{e}")
                     for e in range(EC)]
            u_sb = const.tile([128, EC], F32, name="ub", tag="ub")
            bt_sb = const.tile([128, E], F32, name="btb", tag="btb")
            mask_sb = const.tile([128, 3 * 256], F32, name="msk", tag="msk")
            id_sb = const.tile([128, 128], BF16, name="idn", tag="idn")

            # ---- persistent tensors ----
            xTa = big.tile([128, EC, ROWS], BF16, name="xTa", tag="xTa")
            gT = [big.tile([128, HALF], BF16, name=f"gT{f}", tag=f"gT{f}")
                  for f in range(EC)]
            vt = [big.tile([128, E], BF16, name=f"vt{r}", tag=f"vt{r}")
                  for r in range(NV)]

            # Head DMAs spread across four engine queues (each DMA_DIRECT2D
            # occupies its issuing queue ~0.65us, so serialization on Sync
            # was the old critical path). The first matmul group needs only
            # g0 (0.125MB, gpsimd queue) + xT cols 0..192 (0.19MB, sync).
            XSL = [(0, 384), (384, 640), (640, 1152), (1152, 1664),
                   (1664, ROWS)]
            for c0, c1 in XSL:
                nc.sync.dma_start(out=xTa[:, :, c0:c1],
                                  in_=xT_d[:, :, c0:c1])
            nc.scalar.dma_start(out=g0_sb, in_=g0_d[:, :, :])
            nc.scalar.dma_start(out=u_sb, in_=u_d[:, :])
            for e in range(EC):
                nc.scalar.dma_start(out=g_sb[e][:, 128:512],
                                    in_=g_d[128 * e:128 * (e + 1), 128:512])
            for e in range(EC):
                nc.sync.dma_start(out=wt_sb[e],
                                  in_=wt_d[128 * e:128 * (e + 1), :])
            nc.sync.dma_start(out=bt_sb, in_=bt_d[:, :])
            nc.sync.dma_start(out=mask_sb, in_=mask_d[:, :])
            nc.sync.dma_start(out=id_sb, in_=id_d[:, :])

            with tc.tile_pool(name="pp", bufs=2, space="PSUM") as pp:
                # g-projection: gT[f][:, c] = sum_e G[e,f-chunk]^T xT[e][:, 64+c]
                # (queries only: local rows 64..2112), slice-major so early
                # query columns finish across all f first; slice widths grow
                # so the PE starts while x is still streaming in.
                GSL = [(0, 320), (320, 256), (576, 512), (1088, 512),
                       (1600, 448)]
                for s0, ns in GSL:
                    for f in range(EC):
                        ps = pp.tile([128, 512], F32, name=f"pg{f}_{s0}", tag="pp")
                        for e in range(EC):
                            lhs = (g0_sb[:, e, :] if f == 0
                                   else g_sb[e][:, 128 * f:128 * (f + 1)])
                            nc.tensor.matmul(
                                ps[:, :ns],
                                lhs,
                                xTa[:, e, 64 + s0:64 + s0 + ns],
                                start=(e == 0), stop=(e == EC - 1),
                            )
                        nc.scalar.activation(
                            out=gT[f][:, s0:s0 + ns], in_=ps[:, :ns],
                            func=ACT.Identity, bias=u_sb[:, f:f + 1],
                        )

                def vt_chunk(r):
                    # vt[r] = xT[:, rows r]^T Wt + bt   (natural [rows, feat])
                    ps = pp.tile([128, 512], F32, name=f"pv{r}", tag="pp")
                    for e in range(EC):
                        nc.tensor.matmul(
                            ps[:],
                            xTa[:, e, 128 * r:128 * (r + 1)],
                            wt_sb[e][:],
                            start=(e == 0), stop=(e == EC - 1),
                        )
                    nc.vector.tensor_add(vt[r][:], ps[:], bt_sb[:])

                vt_chunk(0)

                with (
                    tc.tile_pool(name="attn", bufs=3) as attn,
                    tc.tile_pool(name="ps_s", bufs=2, space="PSUM") as ps_s,
                    tc.tile_pool(name="ps_t", bufs=2, space="PSUM") as ps_t,
                    tc.tile_pool(name="ps_a", bufs=2, space="PSUM") as ps_a,
                ):
                    # two-stage software pipeline over the 16 query tiles
                    stage = []  # holds (t, pe_t, rd)

                    def drain(stage_state):
                        t, pe_t, rd = stage_state
                        pT = []
                        for half in (0, 1):
                            tp = ps_t.tile([128, 128], BF16,
                                           name=f"tp{t}_{half}", tag="ps_t")
                            nc.tensor.transpose(
                                tp[:], pe_t[:, 128 * half:128 * (half + 1)],
                                id_sb[:])
                            sb = attn.tile([128, 128], BF16,
                                           name=f"pT{t}_{half}", tag=f"pT{half}")
                            nc.vector.tensor_copy(sb[:], tp[:])
                            pT.append(sb)
                        po = ps_a.tile([128, 512], F32, name=f"po{t}", tag="ps_a")
                        for kc in (0, 1):
                            nc.tensor.matmul(
                                po[:], pT[kc][:], vt[t + kc][:],
                                start=(kc == 0), stop=(kc == 1),
                            )
                        ost = attn.tile([128, 512], BF16, name=f"o{t}", tag="ost")
                        nc.scalar.activation(
                            out=ost[:], in_=po[:], func=ACT.Identity,
                            scale=rd[:, 0:1],
                        )
                        nc.sync.dma_start(
                            out=out_d[128 * t:128 * (t + 1), :], in_=ost[:])

                    for t in range(NT):
                        vt_chunk(t + 1)
                        # scores for tile t: [128 q, 256 keys]
                        ps = ps_s.tile([128, 256], F32, name=f"s{t}", tag="ps_s")
                        for e in range(EC):
                            nc.tensor.matmul(
                                ps[:],
                                gT[e][:, 128 * t:128 * (t + 1)],
                                xTa[:, e, 128 * t:128 * t + 256],
                                start=(e == 0), stop=(e == EC - 1),
                            )
                        mi = 0 if t == 0 else (2 if t == NT - 1 else 1)
                        sm = attn.tile([128, 256], F32, name=f"sm{t}", tag="sm")
                        nc.vector.tensor_add(
                            sm[:], ps[:], mask_sb[:, 256 * mi:256 * (mi + 1)])
                        pe_t = attn.tile([128, 256], BF16, name=f"pe{t}", tag="pe")
                        rs = attn.tile([128, 1], F32, name=f"rs{t}", tag="rs")
                        nc.scalar.activation(out=pe_t[:], in_=sm[:], func=ACT.Exp,
                                             accum_out=rs[:])
                        rd = attn.tile([128, 1], F32, name=f"rd{t}", tag="rd")
                        nc.vector.reciprocal(rd[:], rs[:])
                        if len(stage) == 2:
                            drain(stage.pop(0))
                        stage.append((t, pe_t, rd))
                    for st in stage:
                        drain(st)
    nc.compile()
    return nc


def _get_nc():
    if "nc" not in _NC_CACHE:
        _NC_CACHE["nc"] = _build()
    return _NC_CACHE["nc"]


def _prep_shared(W_qkv, b_qkv, W_out, b_out):
    scale = np.float32(1.0) / np.sqrt(np.float32(E))
    Wq = np.asarray(W_qkv[:, :E], dtype=np.float32)
    Wk = np.asarray(W_qkv[:, E:2 * E], dtype=np.float32)
    Wv = np.asarray(W_qkv[:, 2 * E:], dtype=np.float32)
    bq = np.asarray(b_qkv[:E], dtype=np.float32)
    bv = np.asarray(b_qkv[2 * E:], dtype=np.float32)
    G = (Wq @ Wk.T) * scale                       # [E, E]
    u = (Wk @ bq) * scale                         # [E]
    Wt = Wv @ np.asarray(W_out, dtype=np.float32)  # [E, E]
    bt = bv @ np.asarray(W_out, dtype=np.float32) + np.asarray(
        b_out, dtype=np.float32)                  # [E]
    Gb = G.astype(BF16_NP)
    shared = {
        "gmat": np.ascontiguousarray(Gb),
        "g0": np.ascontiguousarray(
            Gb.reshape(EC, 128, E)[:, :, :128].transpose(1, 0, 2)),
        "wt": np.ascontiguousarray(Wt.astype(BF16_NP)),
        "ubias": np.ascontiguousarray(u.reshape(EC, 128).T.astype(np.float32)),
        "btb": np.ascontiguousarray(
            np.broadcast_to(bt.reshape(1, E), (128, E)).astype(np.float32)),
        "ident": np.eye(128, dtype=np.float32).astype(BF16_NP),
    }
    return shared


def _masks_for(h: int) -> np.ndarray:
    """Additive masks: 0 where attendable, -1e30 outside the band (or past
    the sequence boundary). Columns: [t0 mask | interior mask | t15 mask]."""
    ii = np.arange(128)[:, None]
    jj = np.arange(256)[None, :]
    band = (jj - ii >= 0) & (jj - ii <= 2 * WINDOW)
    m_mid = band
    m_t0 = band & (jj >= 64) if h == 0 else band
    m_t15 = band & (jj < 192) if h == 1 else band
    stacked = np.concatenate([m_t0, m_mid, m_t15], axis=1)
    return np.ascontiguousarray(
        np.where(stacked, np.float32(0.0), np.float32(-1e30)))


def _install_ntff_shim():
    """The agent image's antenv lacks axon_hooks; synthesize it from the
    boot module's ctypes NTFF driver so trace=True can capture HW timing."""
    import types
    if "antenv.axon_hooks" in sys.modules:
        return
    try:
        from trn_agent_boot.trn_boot import _ntff_profile_via_ctypes
        hook = _ntff_profile_via_ctypes("/opt/axon/libaxon_pjrt.so")
    except Exception:
        hook = None
    mod = types.ModuleType("antenv.axon_hooks")
    mod.get_axon_ntff_profile_hook = lambda: hook
    mod.set_axon_ntff_profile_hook = lambda h: None
    sys.modules["antenv.axon_hooks"] = mod
    # avoid S3 artifact upload attempts during local profile processing
    try:
        from concourse import bass_utils as _bu
        _bu.upload_artifacts = lambda tmpdir: tmpdir
    except Exception:
        pass


def kernel(x, W_qkv, b_qkv, W_out, b_out, _trace=False):
    x = np.asarray(x, dtype=np.float32)
    nc = _get_nc()
    shared = _prep_shared(W_qkv, b_qkv, W_out, b_out)
    masks = [_masks_for(0), _masks_for(1)]

    in_maps = []
    for core in range(8):
        b, h = divmod(core, 2)
        lo = h * HALF - WINDOW
        hi = lo + ROWS
        xh = np.zeros((ROWS, E), dtype=np.float32)
        s0, s1 = max(lo, 0), min(hi, S)
        xh[s0 - lo:s1 - lo] = x[b, s0:s1]
        xTp = xh.T.astype(BF16_NP).reshape(EC, 128, ROWS).transpose(1, 0, 2)
        in_maps.append({
            "xTp": np.ascontiguousarray(xTp),
            "masks": masks[h],
            **shared,
        })

    kwargs = {}
    if _trace:
        _install_ntff_shim()
        kwargs = dict(trace=True, trace_cores=[0])
    res = run_bass_kernel_spmd(nc, in_maps, core_ids=list(range(8)), **kwargs)

    out = np.empty((B, S, E), dtype=np.float32)
    for core in range(8):
        b, h = divmod(core, 2)
        out[b, h * HALF:(h + 1) * HALF] = res.results[core]["out"].astype(
            np.float32)
    if _trace:
        return out, res
    return out


# revision 41
# speedup vs baseline: 1.0874x; 1.0236x over previous
"""LocalWindowAttention Trainium2 kernel (Bass/Tile), 8-core SPMD.

Problem: x[B=4, S=4096, E=512] -> out[B, S, E]
  qkv = x @ W_qkv + b_qkv ; q,k,v = split(qkv)
  scores = (q @ k.T) / sqrt(E), banded mask |i-j| <= 64, softmax
  out = (attn @ v) @ W_out + b_out

Sharding: 8 cores = (batch b in 0..3) x (seq half h in 0..1). Each core owns
2048 query rows and loads a 64-row halo of x on each side (zero-padded at
sequence boundaries) -- no collectives.

Algebraic restructure (host-side weight fusion; softmax invariances):
  scores_ij = q_i . k_j / sqrt(E)
            = x_i (Wq Wk^T / sqrt(E)) x_j^T + x_j . (Wk bq / sqrt(E)) + f(i)
  where f(i) terms are constant across the softmax axis and drop. So with
  G = Wq Wk^T / sqrt(E) and u = Wk bq / sqrt(E):
    g = x G + u   (one [S,E]x[E,E] GEMM; k-projection is just x itself)
    scores = g x^T
  Softmax rows sum to 1, so the v/out projections fuse:
    out = attn @ (x (Wv Wout) + (bv Wout + bout)) = attn @ vt
  with Wt = Wv Wout and bt = bv Wout + bout precomputed on host. This
  removes the k-projection, the whole output projection, and both bias
  matmuls: per-core PE work drops from ~190k to ~104k PE columns.

All matmuls run in bf16 (1 PE cycle/column incl. transposes; inputs rounded
on host; fp8 DoubleRow was measured slower than bf16 on HW despite the cost
model). Softmax normalization is applied late (per-partition scale on the
attended output) so the raw exp output feeds the PE transpose directly; the
output is written bf16 and upcast on host.

Schedule: head DMAs split across the SP and Activation queues (DMA issue is
~0.65us per instruction per queue) with x packed [p, e, n] so one DMA per
column-slice covers all 4 contraction chunks. The attention loop is
software-pipelined two stages deep so the PE never waits on the
vector/scalar exp chain, with one vt row-chunk per iteration as extra PE
slack. Measured ~69.1us on HW vs the 134.8us fp32r baseline.
"""

import sys

sys.path.insert(0, "/opt/trn_rl_repo")

import numpy as np
import ml_dtypes

import concourse.bass as bass  # noqa: F401  (registers types)
import concourse.tile as tile
from concourse import bacc, mybir
from concourse.bass_utils import run_bass_kernel_spmd

F32 = mybir.dt.float32
BF16 = mybir.dt.bfloat16
BF16_NP = ml_dtypes.bfloat16

B, S, E = 4, 4096, 512
WINDOW = 64
HALF = S // 2              # 2048 query rows per core
ROWS = HALF + 2 * WINDOW   # 2176 local rows incl. halo
EC = E // 128              # 4 contraction chunks
NT = HALF // 128           # 16 query subtiles per core
NV = ROWS // 128           # 17 vt row chunks

_NC_CACHE = {}


def _build():
    nc = bacc.Bacc("TRN2", target_bir_lowering=False, debug=False, num_devices=8)

    # x packed [p, e, n] so one DMA per column-slice covers all 4 chunks
    xT_d = nc.dram_tensor("xTp", [128, EC, ROWS], BF16, kind="ExternalInput")
    # G's f0 column blocks packed [p, e, m] (the first matmul group's
    # weights in a single 0.125MB DMA with 1KB lines); f1..f3 per chunk
    g0_d = nc.dram_tensor("g0", [128, EC, 128], BF16, kind="ExternalInput")
    g_d = nc.dram_tensor("gmat", [E, E], BF16, kind="ExternalInput")
    wt_d = nc.dram_tensor("wt", [E, E], BF16, kind="ExternalInput")
    u_d = nc.dram_tensor("ubias", [128, EC], F32, kind="ExternalInput")
    bt_d = nc.dram_tensor("btb", [128, E], F32, kind="ExternalInput")
    mask_d = nc.dram_tensor("masks", [128, 3 * 256], F32, kind="ExternalInput")
    id_d = nc.dram_tensor("ident", [128, 128], BF16, kind="ExternalInput")
    out_d = nc.dram_tensor("out", [HALF, E], BF16, kind="ExternalOutput")

    ACT = mybir.ActivationFunctionType

    with tile.TileContext(nc) as tc:
        with (
            tc.tile_pool(name="const", bufs=1) as const,
            tc.tile_pool(name="big", bufs=1) as big,
        ):
            # ---- constants ----
            g0_sb = const.tile([128, EC, 128], BF16, name="gf0", tag="gf0")
            g_sb = [const.tile([128, E], BF16, name=f"g{e}", tag=f"g{e}")
                    for e in range(EC)]
            wt_sb = [const.tile([128, E], BF16, name=f"wt{e}", tag=f"wt{e}")
                     for e in range(EC)]
            u_sb = const.tile([128, EC], F32, name="ub", tag="ub")
            bt_sb = const.tile([128, E], F32, name="btb", tag="btb")
            mask_sb = const.tile([128, 3 * 256], F32, name="msk", tag="msk")
            id_sb = const.tile([128, 128], BF16, name="idn", tag="idn")

            # ---- persistent tensors ----
            xTa = big.tile([128, EC, ROWS], BF16, name="xTa", tag="xTa")
            gT = [big.tile([128, HALF], BF16, name=f"gT{f}", tag=f"gT{f}")
                  for f in range(EC)]
            vt = [big.tile([128, E], BF16, name=f"vt{r}", tag=f"vt{r}")
                  for r in range(NV)]

            # Head DMAs spread across four engine queues (each DMA_DIRECT2D
            # occupies its issuing queue ~0.65us, so serialization on Sync
            # was the old critical path). The first matmul group needs only
            # g0 (0.125MB, gpsimd queue) + xT cols 0..192 (0.19MB, sync).
            XSL = [(0, 384), (384, 640), (640, 1152), (1152, 1664),
                   (1664, ROWS)]
            for c0, c1 in XSL:
                nc.sync.dma_start(out=xTa[:, :, c0:c1],
                                  in_=xT_d[:, :, c0:c1])
            nc.scalar.dma_start(out=g0_sb, in_=g0_d[:, :, :])
            nc.scalar.dma_start(out=u_sb, in_=u_d[:, :])
            for e in range(EC):
                nc.scalar.dma_start(out=g_sb[e][:, 128:512],
                                    in_=g_d[128 * e:128 * (e + 1), 128:512])
            for e in range(EC):
                nc.sync.dma_start(out=wt_sb[e],
                                  in_=wt_d[128 * e:128 * (e + 1), :])
            nc.sync.dma_start(out=bt_sb, in_=bt_d[:, :])
            nc.sync.dma_start(out=mask_sb, in_=mask_d[:, :])
            nc.sync.dma_start(out=id_sb, in_=id_d[:, :])

            with tc.tile_pool(name="pp", bufs=2, space="PSUM") as pp:
                # g-projection: gT[f][:, c] = sum_e G[e,f-chunk]^T xT[e][:, 64+c]
                # (queries only: local rows 64..2112), slice-major so early
                # query columns finish across all f first; slice widths grow
                # so the PE starts while x is still streaming in.
                GSL = [(0, 320), (320, 256), (576, 512), (1088, 512),
                       (1600, 448)]
                for s0, ns in GSL:
                    for f in range(EC):
                        ps = pp.tile([128, 512], F32, name=f"pg{f}_{s0}", tag="pp")
                        for e in range(EC):
                            lhs = (g0_sb[:, e, :] if f == 0
                                   else g_sb[e][:, 128 * f:128 * (f + 1)])
                            nc.tensor.matmul(
                                ps[:, :ns],
                                lhs,
                                xTa[:, e, 64 + s0:64 + s0 + ns],
                                start=(e == 0), stop=(e == EC - 1),
                            )
                        nc.scalar.activation(
                            out=gT[f][:, s0:s0 + ns], in_=ps[:, :ns],
                            func=ACT.Identity, bias=u_sb[:, f:f + 1],
                        )

                def vt_chunk(r):
                    # vt[r] = xT[:, rows r]^T Wt + bt   (natural [rows, feat])
                    ps = pp.tile([128, 512], F32, name=f"pv{r}", tag="pp")
                    for e in range(EC):
                        nc.tensor.matmul(
                            ps[:],
                            xTa[:, e, 128 * r:128 * (r + 1)],
                            wt_sb[e][:],
                            start=(e == 0), stop=(e == EC - 1),
                        )
                    nc.vector.tensor_add(vt[r][:], ps[:], bt_sb[:])

                vt_chunk(0)

                with (
                    tc.tile_pool(name="attn", bufs=3) as attn,
                    tc.tile_pool(name="ps_s", bufs=2, space="PSUM") as ps_s,
                    tc.tile_pool(name="ps_t", bufs=2, space="PSUM") as ps_t,
                    tc.tile_pool(name="ps_a", bufs=2, space="PSUM") as ps_a,
                ):
                    # two-stage software pipeline over the 16 query tiles
                    stage = []  # holds (t, pe_t, rd)

                    def drain(stage_state):
                        t, pe_t, rd = stage_state
                        pT = []
                        for half in (0, 1):
                            tp = ps_t.tile([128, 128], BF16,
                                           name=f"tp{t}_{half}", tag="ps_t")
                            nc.tensor.transpose(
                                tp[:], pe_t[:, 128 * half:128 * (half + 1)],
                                id_sb[:])
                            sb = attn.tile([128, 128], BF16,
                                           name=f"pT{t}_{half}", tag=f"pT{half}")
                            nc.vector.tensor_copy(sb[:], tp[:])
                            pT.append(sb)
                        po = ps_a.tile([128, 512], F32, name=f"po{t}", tag="ps_a")
                        for kc in (0, 1):
                            nc.tensor.matmul(
                                po[:], pT[kc][:], vt[t + kc][:],
                                start=(kc == 0), stop=(kc == 1),
                            )
                        ost = attn.tile([128, 512], BF16, name=f"o{t}", tag="ost")
                        nc.scalar.activation(
                            out=ost[:], in_=po[:], func=ACT.Identity,
                            scale=rd[:, 0:1],
                        )
                        nc.sync.dma_start(
                            out=out_d[128 * t:128 * (t + 1), :], in_=ost[:])

                    for t in range(NT):
                        # scores for tile t: [128 q, 256 keys]
                        ps = ps_s.tile([128, 256], F32, name=f"s{t}", tag="ps_s")
                        for e in range(EC):
                            nc.tensor.matmul(
                                ps[:],
                                gT[e][:, 128 * t:128 * (t + 1)],
                                xTa[:, e, 128 * t:128 * t + 256],
                                start=(e == 0), stop=(e == EC - 1),
                            )
                        mi = 0 if t == 0 else (2 if t == NT - 1 else 1)
                        sm = attn.tile([128, 256], F32, name=f"sm{t}", tag="sm")
                        nc.vector.tensor_add(
                            sm[:], ps[:], mask_sb[:, 256 * mi:256 * (mi + 1)])
                        pe_t = attn.tile([128, 256], BF16, name=f"pe{t}", tag="pe")
                        rs = attn.tile([128, 1], F32, name=f"rs{t}", tag="rs")
                        nc.scalar.activation(out=pe_t[:], in_=sm[:], func=ACT.Exp,
                                             accum_out=rs[:])
                        rd = attn.tile([128, 1], F32, name=f"rd{t}", tag="rd")
                        nc.vector.reciprocal(rd[:], rs[:])
                        if len(stage) == 2:
                            drain(stage.pop(0))
                        vt_chunk(t + 1)
                        stage.append((t, pe_t, rd))
                    for st in stage:
                        drain(st)
    nc.compile()
    return nc


def _get_nc():
    if "nc" not in _NC_CACHE:
        _NC_CACHE["nc"] = _build()
    return _NC_CACHE["nc"]


def _prep_shared(W_qkv, b_qkv, W_out, b_out):
    scale = np.float32(1.0) / np.sqrt(np.float32(E))
    Wq = np.asarray(W_qkv[:, :E], dtype=np.float32)
    Wk = np.asarray(W_qkv[:, E:2 * E], dtype=np.float32)
    Wv = np.asarray(W_qkv[:, 2 * E:], dtype=np.float32)
    bq = np.asarray(b_qkv[:E], dtype=np.float32)
    bv = np.asarray(b_qkv[2 * E:], dtype=np.float32)
    G = (Wq @ Wk.T) * scale                       # [E, E]
    u = (Wk @ bq) * scale                         # [E]
    Wt = Wv @ np.asarray(W_out, dtype=np.float32)  # [E, E]
    bt = bv @ np.asarray(W_out, dtype=np.float32) + np.asarray(
        b_out, dtype=np.float32)                  # [E]
    Gb = G.astype(BF16_NP)
    shared = {
        "gmat": np.ascontiguousarray(Gb),
        "g0": np.ascontiguousarray(
            Gb.reshape(EC, 128, E)[:, :, :128].transpose(1, 0, 2)),
        "wt": np.ascontiguousarray(Wt.astype(BF16_NP)),
        "ubias": np.ascontiguousarray(u.reshape(EC, 128).T.astype(np.float32)),
        "btb": np.ascontiguousarray(
            np.broadcast_to(bt.reshape(1, E), (128, E)).astype(np.float32)),
        "ident": np.eye(128, dtype=np.float32).astype(BF16_NP),
    }
    return shared


def _masks_for(h: int) -> np.ndarray:
    """Additive masks: 0 where attendable, -1e30 outside the band (or past
    the sequence boundary). Columns: [t0 mask | interior mask | t15 mask]."""
    ii = np.arange(128)[:, None]
    jj = np.arange(256)[None, :]
    band = (jj - ii >= 0) & (jj - ii <= 2 * WINDOW)
    m_mid = band
    m_t0 = band & (jj >= 64) if h == 0 else band
    m_t15 = band & (jj < 192) if h == 1 else band
    stacked = np.concatenate([m_t0, m_mid, m_t15], axis=1)
    return np.ascontiguousarray(
        np.where(stacked, np.float32(0.0), np.float32(-1e30)))


def _install_ntff_shim():
    """The agent image's antenv lacks axon_hooks; synthesize it from the
    boot module's ctypes NTFF driver so trace=True can capture HW timing."""
    import types
    if "antenv.axon_hooks" in sys.modules:
        return
    try:
        from trn_agent_boot.trn_boot import _ntff_profile_via_ctypes
        hook = _ntff_profile_via_ctypes("/opt/axon/libaxon_pjrt.so")
    except Exception:
        hook = None
    mod = types.ModuleType("antenv.axon_hooks")
    mod.get_axon_ntff_profile_hook = lambda: hook
    mod.set_axon_ntff_profile_hook = lambda h: None
    sys.modules["antenv.axon_hooks"] = mod
    # avoid S3 artifact upload attempts during local profile processing
    try:
        from concourse import bass_utils as _bu
        _bu.upload_artifacts = lambda tmpdir: tmpdir
    except Exception:
        pass


def kernel(x, W_qkv, b_qkv, W_out, b_out, _trace=False):
    x = np.asarray(x, dtype=np.float32)
    nc = _get_nc()
    shared = _prep_shared(W_qkv, b_qkv, W_out, b_out)
    masks = [_masks_for(0), _masks_for(1)]

    in_maps = []
    for core in range(8):
        b, h = divmod(core, 2)
        lo = h * HALF - WINDOW
        hi = lo + ROWS
        xh = np.zeros((ROWS, E), dtype=np.float32)
        s0, s1 = max(lo, 0), min(hi, S)
        xh[s0 - lo:s1 - lo] = x[b, s0:s1]
        xTp = xh.T.astype(BF16_NP).reshape(EC, 128, ROWS).transpose(1, 0, 2)
        in_maps.append({
            "xTp": np.ascontiguousarray(xTp),
            "masks": masks[h],
            **shared,
        })

    kwargs = {}
    if _trace:
        _install_ntff_shim()
        kwargs = dict(trace=True, trace_cores=[0])
    res = run_bass_kernel_spmd(nc, in_maps, core_ids=list(range(8)), **kwargs)

    out = np.empty((B, S, E), dtype=np.float32)
    for core in range(8):
        b, h = divmod(core, 2)
        out[b, h * HALF:(h + 1) * HALF] = res.results[core]["out"].astype(
            np.float32)
    if _trace:
        return out, res
    return out


# revision 43
# speedup vs baseline: 1.0910x; 1.0033x over previous
"""LocalWindowAttention Trainium2 kernel (Bass/Tile), 8-core SPMD.

Problem: x[B=4, S=4096, E=512] -> out[B, S, E]
  qkv = x @ W_qkv + b_qkv ; q,k,v = split(qkv)
  scores = (q @ k.T) / sqrt(E), banded mask |i-j| <= 64, softmax
  out = (attn @ v) @ W_out + b_out

Sharding: 8 cores = (batch b in 0..3) x (seq half h in 0..1). Each core owns
2048 query rows and loads a 64-row halo of x on each side (zero-padded at
sequence boundaries) -- no collectives.

Algebraic restructure (host-side weight fusion; softmax invariances):
  scores_ij = q_i . k_j / sqrt(E)
            = x_i (Wq Wk^T / sqrt(E)) x_j^T + x_j . (Wk bq / sqrt(E)) + f(i)
  where f(i) terms are constant across the softmax axis and drop. So with
  G = Wq Wk^T / sqrt(E) and u = Wk bq / sqrt(E):
    g = x G + u   (one [S,E]x[E,E] GEMM; k-projection is just x itself)
    scores = g x^T
  Softmax rows sum to 1, so the v/out projections fuse:
    out = attn @ (x (Wv Wout) + (bv Wout + bout)) = attn @ vt
  with Wt = Wv Wout and bt = bv Wout + bout precomputed on host. This
  removes the k-projection, the whole output projection, and both bias
  matmuls: per-core PE work drops from ~190k to ~104k PE columns.

All matmuls run in bf16 (1 PE cycle/column incl. transposes; inputs rounded
on host; fp8 DoubleRow was measured slower than bf16 on HW despite the cost
model). Softmax normalization is applied late (per-partition scale on the
attended output) so the raw exp output feeds the PE transpose directly; the
output is written bf16 and upcast on host.

Schedule: head DMAs split across the SP and Activation queues (DMA issue is
~0.65us per instruction per queue) with x packed [p, e, n] so one DMA per
column-slice covers all 4 contraction chunks. The attention loop is
software-pipelined two stages deep so the PE never waits on the
vector/scalar exp chain, with one vt row-chunk per iteration as extra PE
slack. Measured ~69.1us on HW vs the 134.8us fp32r baseline.
"""

import sys

sys.path.insert(0, "/opt/trn_rl_repo")

import numpy as np
import ml_dtypes

import concourse.bass as bass  # noqa: F401  (registers types)
import concourse.tile as tile
from concourse import bacc, mybir
from concourse.bass_utils import run_bass_kernel_spmd

F32 = mybir.dt.float32
BF16 = mybir.dt.bfloat16
BF16_NP = ml_dtypes.bfloat16

B, S, E = 4, 4096, 512
WINDOW = 64
HALF = S // 2              # 2048 query rows per core
ROWS = HALF + 2 * WINDOW   # 2176 local rows incl. halo
EC = E // 128              # 4 contraction chunks
NT = HALF // 128           # 16 query subtiles per core
NV = ROWS // 128           # 17 vt row chunks

_NC_CACHE = {}


def _build():
    nc = bacc.Bacc("TRN2", target_bir_lowering=False, debug=False, num_devices=8)

    # x packed [p, e, n] so one DMA per column-slice covers all 4 chunks
    xT_d = nc.dram_tensor("xTp", [128, EC, ROWS], BF16, kind="ExternalInput")
    # G's f0 column blocks packed [p, e, m] (the first matmul group's
    # weights in a single 0.125MB DMA with 1KB lines); f1..f3 per chunk
    g0_d = nc.dram_tensor("g0", [128, EC, 128], BF16, kind="ExternalInput")
    g_d = nc.dram_tensor("gmat", [E, E], BF16, kind="ExternalInput")
    wt_d = nc.dram_tensor("wt", [E, E], BF16, kind="ExternalInput")
    u_d = nc.dram_tensor("ubias", [128, EC], F32, kind="ExternalInput")
    bt_d = nc.dram_tensor("btb", [128, E], F32, kind="ExternalInput")
    mask_d = nc.dram_tensor("masks", [128, 3 * 256], F32, kind="ExternalInput")
    id_d = nc.dram_tensor("ident", [128, 128], BF16, kind="ExternalInput")
    out_d = nc.dram_tensor("out", [HALF, E], BF16, kind="ExternalOutput")

    ACT = mybir.ActivationFunctionType

    with tile.TileContext(nc) as tc:
        with (
            tc.tile_pool(name="const", bufs=1) as const,
            tc.tile_pool(name="big", bufs=1) as big,
        ):
            # ---- constants ----
            g0_sb = const.tile([128, EC, 128], BF16, name="gf0", tag="gf0")
            g_sb = [const.tile([128, E], BF16, name=f"g{e}", tag=f"g{e}")
                    for e in range(EC)]
            wt_sb = [const.tile([128, E], BF16, name=f"wt{e}", tag=f"wt{e}")
                     for e in range(EC)]
            u_sb = const.tile([128, EC], F32, name="ub", tag="ub")
            bt_sb = const.tile([128, E], F32, name="btb", tag="btb")
            mask_sb = const.tile([128, 3 * 256], F32, name="msk", tag="msk")
            id_sb = const.tile([128, 128], BF16, name="idn", tag="idn")

            # ---- persistent tensors ----
            xTa = big.tile([128, EC, ROWS], BF16, name="xTa", tag="xTa")
            gT = [big.tile([128, HALF], BF16, name=f"gT{f}", tag=f"gT{f}")
                  for f in range(EC)]
            vt = [big.tile([128, E], BF16, name=f"vt{r}", tag=f"vt{r}")
                  for r in range(NV)]

            # Head DMAs spread across four engine queues (each DMA_DIRECT2D
            # occupies its issuing queue ~0.65us, so serialization on Sync
            # was the old critical path). The first matmul group needs only
            # g0 (0.125MB, gpsimd queue) + xT cols 0..192 (0.19MB, sync).
            XSL = [(0, 384), (384, 640), (640, 1152), (1152, 1664),
                   (1664, ROWS)]
            for c0, c1 in XSL:
                nc.sync.dma_start(out=xTa[:, :, c0:c1],
                                  in_=xT_d[:, :, c0:c1])
            nc.scalar.dma_start(out=g0_sb, in_=g0_d[:, :, :])
            nc.scalar.dma_start(out=u_sb, in_=u_d[:, :])
            for e in range(EC):
                nc.scalar.dma_start(out=g_sb[e][:, 128:512],
                                    in_=g_d[128 * e:128 * (e + 1), 128:512])
            for e in range(EC):
                nc.sync.dma_start(out=wt_sb[e],
                                  in_=wt_d[128 * e:128 * (e + 1), :])
            nc.sync.dma_start(out=bt_sb, in_=bt_d[:, :])
            nc.sync.dma_start(out=mask_sb, in_=mask_d[:, :])
            nc.sync.dma_start(out=id_sb, in_=id_d[:, :])

            with tc.tile_pool(name="pp", bufs=2, space="PSUM") as pp:
                # g-projection: gT[f][:, c] = sum_e G[e,f-chunk]^T xT[e][:, 64+c]
                # (queries only: local rows 64..2112), slice-major so early
                # query columns finish across all f first; slice widths grow
                # so the PE starts while x is still streaming in.
                GSL = [(0, 320), (320, 256), (576, 512), (1088, 512),
                       (1600, 448)]
                for s0, ns in GSL:
                    for f in range(EC):
                        ps = pp.tile([128, 512], F32, name=f"pg{f}_{s0}", tag="pp")
                        for e in range(EC):
                            lhs = (g0_sb[:, e, :] if f == 0
                                   else g_sb[e][:, 128 * f:128 * (f + 1)])
                            nc.tensor.matmul(
                                ps[:, :ns],
                                lhs,
                                xTa[:, e, 64 + s0:64 + s0 + ns],
                                start=(e == 0), stop=(e == EC - 1),
                            )
                        nc.scalar.activation(
                            out=gT[f][:, s0:s0 + ns], in_=ps[:, :ns],
                            func=ACT.Identity, bias=u_sb[:, f:f + 1],
                        )

                def vt_chunk(r):
                    # vt[r] = xT[:, rows r]^T Wt + bt   (natural [rows, feat])
                    ps = pp.tile([128, 512], F32, name=f"pv{r}", tag="pp")
                    for e in range(EC):
                        nc.tensor.matmul(
                            ps[:],
                            xTa[:, e, 128 * r:128 * (r + 1)],
                            wt_sb[e][:],
                            start=(e == 0), stop=(e == EC - 1),
                        )
                    nc.vector.tensor_add(vt[r][:], ps[:], bt_sb[:])

                vt_chunk(0)

                with (
                    tc.tile_pool(name="attn", bufs=3) as attn,
                    tc.tile_pool(name="ps_s", bufs=2, space="PSUM") as ps_s,
                    tc.tile_pool(name="ps_t", bufs=2, space="PSUM") as ps_t,
                    tc.tile_pool(name="ps_a", bufs=2, space="PSUM") as ps_a,
                ):
                    # two-stage software pipeline over the 16 query tiles
                    stage = []  # holds (t, pe_t, rd)

                    def drain(stage_state):
                        t, pe_t, rd = stage_state
                        pT = []
                        for half in (0, 1):
                            tp = ps_t.tile([128, 128], BF16,
                                           name=f"tp{t}_{half}", tag="ps_t")
                            nc.tensor.transpose(
                                tp[:], pe_t[:, 128 * half:128 * (half + 1)],
                                id_sb[:])
                            sb = attn.tile([128, 128], BF16,
                                           name=f"pT{t}_{half}", tag=f"pT{half}")
                            nc.vector.tensor_copy(sb[:], tp[:])
                            pT.append(sb)
                        po = ps_a.tile([128, 512], F32, name=f"po{t}", tag="ps_a")
                        for kc in (0, 1):
                            nc.tensor.matmul(
                                po[:], pT[kc][:], vt[t + kc][:],
                                start=(kc == 0), stop=(kc == 1),
                            )
                        ost = attn.tile([128, 512], BF16, name=f"o{t}", tag="ost")
                        nc.scalar.activation(
                            out=ost[:], in_=po[:], func=ACT.Identity,
                            scale=rd[:, 0:1],
                        )
                        nc.sync.dma_start(
                            out=out_d[128 * t:128 * (t + 1), :], in_=ost[:])

                    for t in range(NT):
                        # scores for tile t: [128 q, 256 keys]
                        ps = ps_s.tile([128, 256], F32, name=f"s{t}", tag="ps_s")
                        for e in range(EC):
                            nc.tensor.matmul(
                                ps[:],
                                gT[e][:, 128 * t:128 * (t + 1)],
                                xTa[:, e, 128 * t:128 * t + 256],
                                start=(e == 0), stop=(e == EC - 1),
                            )
                        mi = 0 if t == 0 else (2 if t == NT - 1 else 1)
                        sm = attn.tile([128, 256], F32, name=f"sm{t}", tag="sm")
                        nc.vector.tensor_add(
                            sm[:], ps[:], mask_sb[:, 256 * mi:256 * (mi + 1)])
                        pe_t = attn.tile([128, 256], BF16, name=f"pe{t}", tag="pe")
                        rs = attn.tile([128, 1], F32, name=f"rs{t}", tag="rs")
                        nc.scalar.activation(out=pe_t[:], in_=sm[:], func=ACT.Exp,
                                             accum_out=rs[:])
                        rd = attn.tile([128, 1], F32, name=f"rd{t}", tag="rd")
                        nc.vector.reciprocal(rd[:], rs[:])
                        if len(stage) == 2:
                            drain(stage.pop(0))
                        vt_chunk(t + 1)
                        stage.append((t, pe_t, rd))
                    for st in stage:
                        drain(st)
    nc.compile()
    return nc


def _get_nc():
    if "nc" not in _NC_CACHE:
        _NC_CACHE["nc"] = _build()
    return _NC_CACHE["nc"]


def _prep_shared(W_qkv, b_qkv, W_out, b_out):
    scale = np.float32(1.0) / np.sqrt(np.float32(E))
    Wq = np.asarray(W_qkv[:, :E], dtype=np.float32)
    Wk = np.asarray(W_qkv[:, E:2 * E], dtype=np.float32)
    Wv = np.asarray(W_qkv[:, 2 * E:], dtype=np.float32)
    bq = np.asarray(b_qkv[:E], dtype=np.float32)
    bv = np.asarray(b_qkv[2 * E:], dtype=np.float32)
    G = (Wq @ Wk.T) * scale                       # [E, E]
    u = (Wk @ bq) * scale                         # [E]
    Wt = Wv @ np.asarray(W_out, dtype=np.float32)  # [E, E]
    bt = bv @ np.asarray(W_out, dtype=np.float32) + np.asarray(
        b_out, dtype=np.float32)                  # [E]
    Gb = G.astype(BF16_NP)
    shared = {
        "gmat": np.ascontiguousarray(Gb),
        "g0": np.ascontiguousarray(
            Gb.reshape(EC, 128, E)[:, :, :128].transpose(1, 0, 2)),
        "wt": np.ascontiguousarray(Wt.astype(BF16_NP)),
        "ubias": np.ascontiguousarray(u.reshape(EC, 128).T.astype(np.float32)),
        "btb": np.ascontiguousarray(
            np.broadcast_to(bt.reshape(1, E), (128, E)).astype(np.float32)),
        "ident": np.eye(128, dtype=np.float32).astype(BF16_NP),
    }
    return shared


def _masks_for(h: int) -> np.ndarray:
    """Additive masks: 0 where attendable, -1e30 outside the band (or past
    the sequence boundary). Columns: [t0 mask | interior mask | t15 mask]."""
    ii = np.arange(128)[:, None]
    jj = np.arange(256)[None, :]
    band = (jj - ii >= 0) & (jj - ii <= 2 * WINDOW)
    m_mid = band
    m_t0 = band & (jj >= 64) if h == 0 else band
    m_t15 = band & (jj < 192) if h == 1 else band
    stacked = np.concatenate([m_t0, m_mid, m_t15], axis=1)
    return np.ascontiguousarray(
        np.where(stacked, np.float32(0.0), np.float32(-1e30)))


def _install_ntff_shim():
    """The agent image's antenv lacks axon_hooks; synthesize it from the
    boot module's ctypes NTFF driver so trace=True can capture HW timing."""
    import types
    if "antenv.axon_hooks" in sys.modules:
        return
    try:
        from trn_agent_boot.trn_boot import _ntff_profile_via_ctypes
        hook = _ntff_profile_via_ctypes("/opt/axon/libaxon_pjrt.so")
    except Exception:
        hook = None
    mod = types.ModuleType("antenv.axon_hooks")
    mod.get_axon_ntff_profile_hook = lambda: hook
    mod.set_axon_ntff_profile_hook = lambda h: None
    sys.modules["antenv.axon_hooks"] = mod
    # avoid S3 artifact upload attempts during local profile processing
    try:
        from concourse import bass_utils as _bu
        _bu.upload_artifacts = lambda tmpdir: tmpdir
    except Exception:
        pass


def kernel(x, W_qkv, b_qkv, W_out, b_out, _trace=False):
    x = np.asarray(x, dtype=np.float32)
    nc = _get_nc()
    shared = _prep_shared(W_qkv, b_qkv, W_out, b_out)
    masks = [_masks_for(0), _masks_for(1)]

    in_maps = []
    for core in range(8):
        b, h = divmod(core, 2)
        lo = h * HALF - WINDOW
        hi = lo + ROWS
        xh = np.zeros((ROWS, E), dtype=np.float32)
        s0, s1 = max(lo, 0), min(hi, S)
        xh[s0 - lo:s1 - lo] = x[b, s0:s1]
        xTp = xh.T.astype(BF16_NP).reshape(EC, 128, ROWS).transpose(1, 0, 2)
        in_maps.append({
            "xTp": np.ascontiguousarray(xTp),
            "masks": masks[h],
            **shared,
        })

    kwargs = {}
    if _trace:
        _install_ntff_shim()
        kwargs = dict(trace=True, trace_cores=[0])
    res = run_bass_kernel_spmd(nc, in_maps, core_ids=list(range(8)), **kwargs)

    out = np.empty((B, S, E), dtype=np.float32)
    for core in range(8):
        b, h = divmod(core, 2)
        out[b, h * HALF:(h + 1) * HALF] = res.results[core]["out"].astype(
            np.float32)
    if _trace:
        return out, res
    return out
